# revision 1
# baseline (speedup 1.0000x reference)
"""Trainium2 Bass kernel for nn_CompositeEmbeddingA (octree composite embedding).

Per sample (1 sample per NeuronCore, batch=8 over 8 cores):
  layers 0-2 (depths 1-3): x = val_emb[v] + pos0[p0] + pos1[p1] + pos2[p2] + dep_emb[d]
  layers 3-4: same sum w/o dep, then Conv1d(E,E,kernel=stride=k), k=4 (l3) / 8 (l4)

Algorithm: every layer is expressed as  out = MultiHot @ Table  on the PE:
  - conv folded into the tables host-side: per tap j, T_j = concat(tables) @ w[:,:,j].T,
    so out[t] = sum_j multihot(token 8t+j) @ T_j  == one K=(196k) matmul per layer.
  - MultiHot^T (contraction dim on partitions) is built on-chip:
      PE "broadcast matmul": bcast[r_row, tok] = selector^T @ idx_rows  (replicates the
      right index value into every table row), then DVE is_equal against a per-partition
      constant column -> exact 0/1 one-hot, fp32.
  - conv bias = one extra table row whose selector column is all-zero (bcast value 0)
    with compare const 0 -> fires for every token.
  - main matmuls run in float32r (full fp32 data, 1 cycle/row at N>=256).
"""

import sys

for _p in ("/opt/trn_rl_repo",):
    if _p not in sys.path:
        sys.path.insert(0, _p)

import numpy as np
import ml_dtypes

RES = 32
SPATIAL = 3
NUM_VOCAB = 3
E = 256
BATCH = 8
LAYER_SIZES = (8, 64, 512, 4096, 32768)
CONV_SIZE = {3: 4, 4: 8}
S_TOTAL = sum(LAYER_SIZES)  # 37448
OUT_TOKENS = 8 + 64 + 512 + 1024 + 4096  # 5704
NIDX = 33  # 32 idx rows + one all-ones row (carries the -c compare constants)
ONES_ROW = 32
STRIPE = 512

# segment widths inside one tap: value(4), pos0(64), pos1(64), pos2(64) [, dep(6)]
SEG_W = (NUM_VOCAB + 1, 2 * RES, 2 * RES, 2 * RES)
DEP_W = 6

_BF16 = ml_dtypes.bfloat16


def _layer_slices():
    out = []
    start = 0
    for n in LAYER_SIZES:
        out.append((start, start + n))
        start += n
    return out


LAYER_SL = _layer_slices()


def _build_consts(params):
    """Fold conv weights into tables; pack rows into 128-row chunks.

    Returns (tbl [NC,128,256] f32, sel [NC,32,128] bf16, cval [NC,128,1] f32,
             layers: list of (name, T_tokens, out_offset, chunk_index_list))
    """
    rows_tbl = []   # per logical row: the 256-vector
    rows_ridx = []  # which of the 32 idx rows feeds this row (-1 = none: bcast val 0)
    rows_c = []     # compare constant
    layer_marks = []  # (row_start, row_end) per virtual layer

    def seg_tables(l):
        t = [np.asarray(params[f"val_emb_{l}"], np.float32)]
        pe = np.asarray(params[f"pos_emb_{l}"], np.float32)
        t += [pe[0], pe[1], pe[2]]
        return t

    # virtual layer "B": real layers 0..2 merged. idx rows: l*5 + (v,p0,p1,p2,d)
    r0 = len(rows_tbl)
    for l in range(3):
        tabs = seg_tables(l) + [np.asarray(params[f"dep_emb_{l}"], np.float32)]
        for seg, tab in enumerate(tabs):
            for c in range(tab.shape[0]):
                rows_tbl.append(tab[c])
                rows_ridx.append(l * 5 + seg)
                rows_c.append(float(c))
    layer_marks.append((r0, len(rows_tbl)))

    # conv layers: idx rows j*4+seg; one bias row (all-zero selector col, c=0)
    for l in (3, 4):
        r0 = len(rows_tbl)
        k = CONV_SIZE[l]
        w = np.asarray(params[f"conv_w_{l}"], np.float32)  # [O, E, k]
        b = np.asarray(params[f"conv_b_{l}"], np.float32)  # [O]
        tabs = seg_tables(l)
        for j in range(k):
            wj = w[:, :, j]  # [O, E]
            for seg, tab in enumerate(tabs):
                folded = tab @ wj.T  # [rows, O]
                for c in range(tab.shape[0]):
                    rows_tbl.append(folded[c])
                    rows_ridx.append(j * 4 + seg)
                    rows_c.append(float(c))
        rows_tbl.append(b)
        rows_ridx.append(-1)
        rows_c.append(0.0)
        layer_marks.append((r0, len(rows_tbl)))

    # chunkify each virtual layer into 128-row chunks
    tbl_chunks, sel_chunks, cval_chunks = [], [], []
    layers = []
    out_offs = [0, 584, 1608]
    names = ["B", "L3", "L4"]
    t_counts = [584, 1024, 4096]
    for vl, (r0, r1) in enumerate(layer_marks):
        n = r1 - r0
        nch = -(-n // 128)
        cids = []
        for ci in range(nch):
            a = r0 + ci * 128
            bnd = min(r0 + (ci + 1) * 128, r1)
            rows = bnd - a
            tbl = np.zeros((128, E), np.float32)
            sel = np.zeros((NIDX, 128), np.float32)
            sel[ONES_ROW, :] = 1.0  # pad rows: bcast value = +1 -> eq(.,0)=0
            for m in range(rows):
                tbl[m] = rows_tbl[a + m]
                if rows_ridx[a + m] >= 0:
                    sel[rows_ridx[a + m], m] = 1.0
                # ones-row coefficient: broadcast out = idx - c
                sel[ONES_ROW, m] = -rows_c[a + m]
            cids.append(len(tbl_chunks))
            tbl_chunks.append(tbl)
            sel_chunks.append(sel.astype(_BF16))
        layers.append((names[vl], t_counts[vl], out_offs[vl], cids))

    # merged layouts: one DMA per constant tensor
    tbl = np.concatenate(tbl_chunks, axis=1)  # [128, NC*256] f32
    sel = np.concatenate(sel_chunks, axis=1)  # [33, NC*128] bf16
    return tbl, sel, layers


def _build_ridx(value, depth, position, b):
    """Per-core index-row tensors, one per virtual layer: [32, T] bf16."""
    out = {}
    # B: merged layers 0-2; out tokens 0..583 = input tokens 0..583
    rb = np.full((NIDX, 584), -1.0, np.float32)
    rb[ONES_ROW] = 1.0
    col = 0
    for l in range(3):
        lo, hi = LAYER_SL[l]
        n = hi - lo
        rb[l * 5 + 0, col : col + n] = value[b, lo:hi]
        for s in range(3):
            rb[l * 5 + 1 + s, col : col + n] = position[b, lo:hi, s]
        rb[l * 5 + 4, col : col + n] = depth[b, lo:hi]
        col += n
    out["B"] = rb.astype(_BF16)
    for name, l in (("L3", 3), ("L4", 4)):
        k = CONV_SIZE[l]
        lo, hi = LAYER_SL[l]
        T = (hi - lo) // k
        r = np.zeros((NIDX, T), np.float32)
        r[ONES_ROW] = 1.0
        for j in range(k):
            r[j * 4 + 0] = value[b, lo:hi][j::k]
            for s in range(3):
                r[j * 4 + 1 + s] = position[b, lo:hi, s][j::k]
        out[name] = r.astype(_BF16)
    return out


_CACHE = {}

# schedule tuning knobs (sweepable via analyze_sweep.py)
PAIR = 1  # chunks fused per eq op
BPS_BUFS = 5
OPS_BUFS = 3
MH_BUFS = 3
ACT_MOD = 4  # pair p goes to ACT when p % ACT_MOD == ACT_MOD - 1
DEPTH = 2
STAGE = "full"  # "full" | "mh_only" | "main_only" (HW bisection)
EQ_BF16 = False  # bf16 PSUM matmul output is TRN3-only
TT_PAIR = 1  # main t-tiles packed per PSUM bank (2 regressed on HW: 311us)


def _get_nc(layers, nchunks, reps=1):
    key = ("v1", PAIR, BPS_BUFS, OPS_BUFS, MH_BUFS, ACT_MOD, DEPTH, reps, STAGE,
           EQ_BF16, TT_PAIR, tuple((n, t, o, tuple(c)) for n, t, o, c in layers))
    if key in _CACHE:
        return _CACHE[key]

    import concourse.bass as bass
    import concourse.tile as tile
    from concourse import bacc, mybir
    from contextlib import ExitStack

    f32 = mybir.dt.float32
    f32r = mybir.dt.float32r
    bf16 = mybir.dt.bfloat16

    nc = bacc.Bacc(trn_type="TRN2", target_bir_lowering=False, debug=False)
    tbl_d = nc.dram_tensor("tbl", [128, nchunks * E], f32r, kind="ExternalInput").ap()
    sel_d = nc.dram_tensor(
        "sel", [NIDX, nchunks * 128], bf16, kind="ExternalInput"
    ).ap()
    ridx_d = {
        name: nc.dram_tensor(f"ridx_{name}", [NIDX, T], bf16, kind="ExternalInput").ap()
        for name, T, _, _ in layers
    }
    out_d = nc.dram_tensor("out", [OUT_TOKENS, E], f32, kind="ExternalOutput").ap()

    with tile.TileContext(nc) as tc, ExitStack() as ctx:
        cpool = ctx.enter_context(tc.tile_pool(name="const", bufs=1))
        rpool = ctx.enter_context(tc.tile_pool(name="ridx", bufs=DEPTH + 1))
        mpool = ctx.enter_context(tc.tile_pool(name="mh", bufs=MH_BUFS))
        tpool = ctx.enter_context(tc.tile_pool(name="sq", bufs=3))
        bps = ctx.enter_context(
            tc.tile_pool(name="bps", bufs=BPS_BUFS, space=bass.MemorySpace.PSUM)
        )
        ops = ctx.enter_context(
            tc.tile_pool(name="ops", bufs=OPS_BUFS, space=bass.MemorySpace.PSUM)
        )
        opool = ctx.enter_context(tc.tile_pool(name="osb", bufs=3))

        # small consts first so the first broadcast matmuls start immediately;
        # the big table load is split per-layer in use order behind them
        sel_t = cpool.tile([NIDX, nchunks * 128], bf16, tag="sel")
        nc.sync.dma_start(sel_t[:], sel_d[:])
        tbl_t = cpool.tile([128, nchunks * E], f32r, tag="tbl")
        for _, _, _, cids in layers:
            lo, hi = cids[0] * E, (cids[-1] + 1) * E
            nc.sync.dma_start(tbl_t[:, lo:hi], tbl_d[:, lo:hi])

        A = mybir.ActivationFunctionType
        stripes = []
        for name, T, out_off, cids in layers:
            for s0 in range(0, T, STRIPE):
                stripes.append((name, out_off, cids, s0, min(STRIPE, T - s0)))
        # spread the small eq-heavy stripes (B/L3) between PE-heavy L4 ones
        big = [s for s in stripes if s[0] == "L4"]
        small = [s for s in stripes if s[0] != "L4"]
        small.sort(key=lambda s: -s[4])  # tiny tail stripe goes last
        stripes = []
        for i, b in enumerate(big):
            stripes.append(b)
            if i * len(small) // len(big) < (i + 1) * len(small) // len(big):
                stripes.append(small[i * len(small) // len(big)])

        def load_ridx(si):
            name, _, cids, s0, W = stripes[si]
            rt = rpool.tile([NIDX, W], bf16, tag="r")
            nc.sync.dma_start(rt[:], ridx_d[name][:, s0 : s0 + W])
            return rt

        def build_mh_pair(si, rt, p, ks):
            """broadcast matmuls + eq for a pair (or single) of chunks.

            The broadcast output is already idx - c (ones-row trick), so the
            one-hot is a compare against immediate 0 and one DVE/ACT op can
            span both chunks of the pair.
            """
            _, _, cids, _, W = stripes[si]
            n = len(ks)
            bp = bps.tile([128, n * W], bf16 if EQ_BF16 else f32, tag="b")
            for i, k in enumerate(ks):
                ci = cids[k]
                nc.tensor.matmul(
                    bp[:, i * W : (i + 1) * W],
                    sel_t[:, ci * 128 : (ci + 1) * 128],
                    rt[:],
                    start=True,
                    stop=True,
                )
            mh = mpool.tile([128, n * W], f32r, tag=f"mh{p}")
            if p % ACT_MOD == ACT_MOD - 1:
                # ACT path: relu(1 - x^2) — exact 0/1 for integer x
                tmp = tpool.tile([128, n * W], f32, tag="sq")
                nc.scalar.activation(tmp[:], bp[:], A.Square)
                nc.scalar.activation(mh[:], tmp[:], A.Relu, bias=1.0, scale=-1.0)
            else:
                nc.vector.tensor_scalar(
                    mh[:], bp[:], 0.0, None, op0=mybir.AluOpType.is_equal
                )
            return [mh[:, i * W : (i + 1) * W] for i in range(n)]

        def main_ttile(si, mhs, ti, ob):
            """two t-tiles packed into one PSUM bank; one evict per pair."""
            _, _, cids, _, W = stripes[si]
            nt = min(TT_PAIR, -(-W // 128) - TT_PAIR * ti)
            op = ops.tile([128, nt * E], f32, tag="o")
            Ms = []
            for h in range(nt):
                t0 = (TT_PAIR * ti + h) * 128
                M = min(128, W - t0)
                Ms.append(M)
                for k, ci in enumerate(cids):
                    nc.tensor.matmul(
                        op[:M, h * E : h * E + E],
                        mhs[k][:, t0 : t0 + M],
                        tbl_t[:, ci * E : (ci + 1) * E],
                        start=(k == 0),
                        stop=(k == len(cids) - 1),
                    )
            col = TT_PAIR * ti * E
            if nt == 2 and Ms[0] == 128 and Ms[1] == 128:
                nc.scalar.activation(ob[:, col : col + 2 * E], op[:], A.Copy)
            else:
                for h in range(nt):
                    nc.scalar.activation(
                        ob[: Ms[h], col + h * E : col + (h + 1) * E],
                        op[: Ms[h], h * E : h * E + E],
                        A.Copy,
                    )

        def store_out(si, ob):
            _, out_off, _, s0, W = stripes[si]
            row = out_off + s0
            if W % 128 == 0:
                dst = out_d[row : row + W, :].rearrange("(a p) e -> p a e", p=128)
                src = ob[:].rearrange("p (a e) -> p a e", e=E)
                nc.sync.dma_start(dst, src)
            else:
                nc.sync.dma_start(out_d[row : row + W, :], ob[:W, :E])

        # two-stripe software pipeline with interleaved emission: pair-builds
        # of stripe s+2's one-hots alternate with stripe s's main t-tiles.
        def stripe_pairs(si):
            nk = len(stripes[si][2])
            return [tuple(range(a, min(a + PAIR, nk))) for a in range(0, nk, PAIR)]

        def emit_pairs(si, rt, prs):
            mhs = []
            for p, ks in prs:
                mhs += build_mh_pair(si, rt, p, ks)
            return mhs

        def emit_body_mh_only():
            for si in range(len(stripes)):
                rt = load_ridx(si)
                emit_pairs(si, rt, list(enumerate(stripe_pairs(si))))

        static_mh = {}
        if STAGE == "main_only":
            tmp0 = cpool.tile([128, STRIPE], f32, tag="smhtmp")
            nc.gpsimd.memset(tmp0[:], 0.5)
            for p in range(13):
                t = cpool.tile([128, STRIPE], f32r, tag=f"smh{p}")
                nc.vector.tensor_scalar(
                    t[:], tmp0[:], 0.0, None, op0=mybir.AluOpType.is_equal
                )
                static_mh[p] = t

        def emit_body_main_only():
            for si in range(len(stripes)):
                _, _, cids, _, W = stripes[si]
                ntt = -(-W // 128)
                ob = opool.tile([128, ntt * E], f32, tag="ob")
                mhs = [static_mh[k][:, :W] for k in range(len(cids))]
                for ti in range(-(-ntt // TT_PAIR)):
                    main_ttile(si, mhs, ti, ob)
                store_out(si, ob)

        def emit_body():
            nst = len(stripes)
            mh_of = {}
            for si in range(min(DEPTH, nst)):
                rt = load_ridx(si)
                mh_of[si] = emit_pairs(si, rt, list(enumerate(stripe_pairs(si))))
            for si in range(nst):
                W = stripes[si][4]
                ntt = -(-W // 128)
                ngr = -(-ntt // TT_PAIR)
                ob = opool.tile([128, ntt * E], f32, tag="ob")
                sj = si + DEPTH
                if sj < nst:
                    rt = load_ridx(sj)
                    prs = list(enumerate(stripe_pairs(sj)))
                    npr = len(prs)
                    # split stripe sj's pair-builds into groups interleaved
                    # with stripe si's main t-tile pairs
                    bounds = [round(g * npr / ngr) for g in range(ngr + 1)]
                    mh_of[sj] = []
                    for ti in range(ngr):
                        main_ttile(si, mh_of[si], ti, ob)
                        mh_of[sj] += emit_pairs(
                            sj, rt, prs[bounds[ti] : bounds[ti + 1]]
                        )
                else:
                    for ti in range(ngr):
                        main_ttile(si, mh_of[si], ti, ob)
                store_out(si, ob)
                del mh_of[si]

        body_fn = {
            "full": emit_body,
            "mh_only": emit_body_mh_only,
            "main_only": emit_body_main_only,
        }[STAGE]
        if reps == 1:
            body_fn()
        else:
            # timing mode: repeat the body on-device to measure per-iter HW
            # time as a wall-clock slope (no NTFF profiling available)
            hints = (
                mybir.EngineType.PE,
                mybir.EngineType.DVE,
                mybir.EngineType.Activation,
                mybir.EngineType.SP,
            )
            with tc.For_i(0, reps, 1, hint_engines=hints):
                body_fn()

    nc.compile()
    _CACHE[key] = nc
    return nc


def kernel(**inputs):
    from concourse.bass_utils import run_bass_kernel_spmd

    value = np.asarray(inputs["value"], np.int32).astype(np.float32)
    depth = np.asarray(inputs["depth"], np.int32).astype(np.float32)
    position = np.asarray(inputs["position"], np.int32).astype(np.float32)

    tbl, sel, layers = _build_consts(inputs)
    nc = _get_nc(layers, tbl.shape[1] // E)

    in_maps = []
    for b in range(BATCH):
        rid = _build_ridx(value, depth, position, b)
        m = {"tbl": tbl, "sel": sel}
        for name, _, _, _ in layers:
            m[f"ridx_{name}"] = rid[name]
        in_maps.append(m)

    res = run_bass_kernel_spmd(nc, in_maps, list(range(BATCH)))
    return np.stack([res.results[b]["out"] for b in range(BATCH)])



# revision 3
# speedup vs baseline: 2.6150x; 2.6150x over previous
"""Trainium2 Bass kernel for nn_CompositeEmbeddingA (octree composite embedding).

Per sample (1 sample per NeuronCore, batch=8 over 8 cores):
  layers 0-2 (depths 1-3): x = val_emb[v] + pos0[p0] + pos1[p1] + pos2[p2] + dep_emb[d]
  layers 3-4: same sum w/o dep, then Conv1d(E,E,kernel=stride=k), k=4 (l3) / 8 (l4)

Design (v2): every layer is out = MultiHot @ Table on the PE, with
  - conv folded into the tables host-side (per tap j, T_j = concat(tables) @ w[:,:,j].T),
  - the multi-hot matrices built ON THE HOST and shipped as fp8 (exact 0/1),
    so the PE does only the main matmuls and DVE/ACT only PSUM evictions,
  - fp8e4m3 DoubleRow matmuls: 2 chunks of 128 table rows contracted per
    instruction at 0.5 cycles/row; tables stored as scaled hi+lo fp8 pairs
    (quantization residual correction) and the eviction rescales by 1/S,
  - layers 0+1 (72 tokens) use one precomputed row per token instead of
    table rows (host computes those 72 sums directly),
  - output written as bf16 and upcast on the host.
"""

import sys

for _p in ("/opt/trn_rl_repo",):
    if _p not in sys.path:
        sys.path.insert(0, _p)

import numpy as np
import ml_dtypes

E = 256
BATCH = 8
LAYER_SIZES = (8, 64, 512, 4096, 32768)
CONV_SIZE = {3: 4, 4: 8}
S_TOTAL = sum(LAYER_SIZES)

F8 = ml_dtypes.float8_e4m3
BF16 = ml_dtypes.bfloat16
F8_ONE = np.asarray(1.0, F8).view(np.uint8).item()
F8_MAX = float(ml_dtypes.finfo(F8).max)

# virtual layers: B = real layers 0-2 merged; L3/L4 conv layers.
#   B: 584 out tokens padded to 640 (5 ttiles); rows = 72 per-token rows
#      (l0+l1) + l2 table (3 val + 189 pos + dep uniques) -> 4 chunks, 2 pairs
#   L3: 1024 tokens (8 tt); 4 taps x 192 rows = 768 -> 6 chunks, 3 pairs
#   L4: 4096 tokens (32 tt); 8 taps x 192 rows = 1536 -> 12 chunks, 6 pairs
_L = [
    dict(name="B", T=584, Tp=640, ntt=5, nch=4),
    dict(name="L3", T=1024, Tp=1024, ntt=8, nch=6),
    dict(name="L4", T=4096, Tp=4096, ntt=32, nch=12),
]
_mhb = 0
_cb = 0
_orow = 0
for _d in _L:
    _d["mh_base"] = _mhb
    _d["cb"] = _cb
    _d["out_row0"] = _orow
    _d["npairs"] = _d["nch"] // 2
    _mhb += _d["ntt"] * _d["nch"] * 128
    _cb += _d["nch"]
    _orow += _d["Tp"]
NCH = _cb
MH_TOTAL = _mhb
OUT_ROWS = _orow  # 5760

# schedule knobs
MH_PIECE_TT = 4  # ttiles per mh DMA piece
STORE_TT = 8  # ttiles per output store DMA
EVICT_PAT = ("dve", "act")  # round robin eviction engines (gpsimd can't read PSUM)


def _build_tables(params):
    """Folded f32 tables per virtual layer (core-independent parts).

    Returns {layer_name: rows [nrows, E] f32}, without B's per-token rows
    (those are per-core, rows 0..71 of B).
    """
    out = {}
    # B: l2 table rows at offset 72: 3 val + 189 pos + dep uniques (built later)
    v2 = np.asarray(params["val_emb_2"], np.float32)[1:4]
    pe2 = np.asarray(params["pos_emb_2"], np.float32)
    out["B_l2"] = np.concatenate([v2, pe2[0][1:64], pe2[1][1:64], pe2[2][1:64]], 0)
    for name, l in (("L3", 3), ("L4", 4)):
        k = CONV_SIZE[l]
        w = np.asarray(params[f"conv_w_{l}"], np.float32)
        b = np.asarray(params[f"conv_b_{l}"], np.float32)
        pe = np.asarray(params[f"pos_emb_{l}"], np.float32)
        base = np.concatenate(
            [
                np.asarray(params[f"val_emb_{l}"], np.float32)[1:4],
                pe[0][1:64],
                pe[1][1:64],
                pe[2][1:64],
            ],
            0,
        )  # [192, E]
        taps = []
        for j in range(k):
            f = base @ w[:, :, j].T
            if j == 0:
                f[:3] += b  # bias fires exactly once per token via the val row
            taps.append(f)
        out[name] = np.concatenate(taps, 0)  # [192k, E]
    return out


def _pack_chunks(rows, nch):
    """[nrows<=nch*128, E] f32 -> [128, nch*E] (partition = row-within-chunk)."""
    buf = np.zeros((nch * 128, E), np.float32)
    buf[: rows.shape[0]] = rows
    return np.ascontiguousarray(
        buf.reshape(nch, 128, E).transpose(1, 0, 2)
    ).reshape(128, nch * E)


def _quant_hilo(packed, S):
    hi = (packed * S).astype(F8)
    lo = (packed * S - hi.astype(np.float32)).astype(F8)
    return hi, lo


def _build_mh(value, depth, position, b, dep2_uniq):
    """Host-built multi-hot for core b: [128, MH_TOTAL] uint8 (fp8 bits)."""
    pieces = []

    def emit(r_ids, t_ids, d):
        M = np.zeros(d["nch"] * 128 * d["Tp"], np.uint8)
        M[r_ids * d["Tp"] + t_ids] = F8_ONE
        M = (
            M.reshape(d["nch"], 128, d["ntt"], 128)
            .transpose(1, 2, 0, 3)
            .reshape(128, -1)
        )
        pieces.append(M)

    # --- B ---
    d = _L[0]
    t01 = np.arange(72)
    v2 = value[b, 72:584]
    p2 = position[b, 72:584]
    d2 = depth[b, 72:584]
    t2 = np.arange(72, 584)
    dep_rows = 264 + np.searchsorted(dep2_uniq, d2)
    r_ids = np.concatenate(
        [
            t01,
            72 + (v2 - 1),
            75 + (p2[:, 0] - 1),
            138 + (p2[:, 1] - 1),
            201 + (p2[:, 2] - 1),
            dep_rows,
        ]
    )
    t_ids = np.concatenate([t01, t2, t2, t2, t2, t2])
    emit(r_ids, t_ids, d)

    # --- conv layers ---
    lo = 584
    for d, l in ((_L[1], 3), (_L[2], 4)):
        k = CONV_SIZE[l]
        T = d["T"]
        v = value[b, lo : lo + T * k].reshape(T, k)
        p = position[b, lo : lo + T * k].reshape(T, k, 3)
        t = np.broadcast_to(np.arange(T)[:, None], (T, k))
        jb = np.broadcast_to(np.arange(k)[None, :] * 192, (T, k))
        r_ids = np.concatenate(
            [
                (jb + v - 1).ravel(),
                (jb + 3 + p[:, :, 0] - 1).ravel(),
                (jb + 66 + p[:, :, 1] - 1).ravel(),
                (jb + 129 + p[:, :, 2] - 1).ravel(),
            ]
        )
        t_ids = np.concatenate([t.ravel()] * 4)
        emit(r_ids, t_ids, d)
        lo += T * k

    return np.concatenate(pieces, axis=1)


_CACHE = {}


def _get_nc(inv_scales):
    key = ("v2", tuple(inv_scales), MH_PIECE_TT, STORE_TT, EVICT_PAT)
    if key in _CACHE:
        return _CACHE[key]

    import concourse.bass as bass
    import concourse.tile as tile
    from concourse import bacc, mybir
    from contextlib import ExitStack

    f32 = mybir.dt.float32
    bf16 = mybir.dt.bfloat16
    f8 = mybir.dt.float8e4
    A = mybir.ActivationFunctionType
    DR = mybir.MatmulPerfMode.DoubleRow

    nc = bacc.Bacc(trn_type="TRN2", target_bir_lowering=False, debug=False)
    mh_d = nc.dram_tensor("mh", [128, MH_TOTAL], f8, kind="ExternalInput").ap()
    tbh_d = nc.dram_tensor("tbh", [128, NCH * E], f8, kind="ExternalInput").ap()
    tbl_d = nc.dram_tensor("tbl", [128, NCH * E], f8, kind="ExternalInput").ap()
    out_d = nc.dram_tensor("out", [OUT_ROWS, E], bf16, kind="ExternalOutput").ap()

    with tile.TileContext(nc) as tc, ExitStack() as ctx:
        cpool = ctx.enter_context(tc.tile_pool(name="const", bufs=1))
        pspool = ctx.enter_context(
            tc.tile_pool(name="ps", bufs=8, space=bass.MemorySpace.PSUM)
        )
        spool = ctx.enter_context(tc.tile_pool(name="stage", bufs=1))

        tbh_t = cpool.tile([128, NCH * E], f8, tag="tbh")
        tbl_t = cpool.tile([128, NCH * E], f8, tag="tbl")
        mh_t = cpool.tile([128, MH_TOTAL], f8, tag="mh")

        # loads in consumption order (SP queue)
        for d in _L:
            ca, cb_ = d["cb"] * E, (d["cb"] + d["nch"]) * E
            nc.sync.dma_start(tbh_t[:, ca:cb_], tbh_d[:, ca:cb_])
            nc.sync.dma_start(tbl_t[:, ca:cb_], tbl_d[:, ca:cb_])
            step = MH_PIECE_TT * d["nch"] * 128
            for a in range(d["mh_base"], d["mh_base"] + d["ntt"] * d["nch"] * 128, step):
                bnd = min(a + step, d["mh_base"] + d["ntt"] * d["nch"] * 128)
                nc.sync.dma_start(mh_t[:, a:bnd], mh_d[:, a:bnd])

        # compute
        ev = 0
        for li, d in enumerate(_L):
            inv_s = inv_scales[li]
            ngroups = -(-d["ntt"] // STORE_TT)
            for g in range(ngroups):
                g0 = g * STORE_TT
                gn = min(STORE_TT, d["ntt"] - g0)
                stage = spool.tile([128, gn * E], bf16, tag=f"st{li}g{g}")
                for ti in range(gn):
                    tt = g0 + ti
                    ps = pspool.tile([128, E], f32, tag="ps")
                    nmm = 2 * d["npairs"]
                    i = 0
                    for q in range(d["npairs"]):
                        ma = d["mh_base"] + (tt * d["nch"] + 2 * q) * 128
                        mh_ap = mh_t[:, ma : ma + 256].rearrange(
                            "p (two m) -> p two m", two=2
                        )
                        ta = (d["cb"] + 2 * q) * E
                        for tb_t in (tbh_t, tbl_t):
                            nc.tensor.matmul(
                                ps[:],
                                mh_ap,
                                tb_t[:, ta : ta + 2 * E].rearrange(
                                    "p (two e) -> p two e", two=2
                                ),
                                start=(i == 0),
                                stop=(i == nmm - 1),
                                perf_mode=DR,
                            )
                            i += 1
                    dst = stage[:, ti * E : (ti + 1) * E]
                    eng = EVICT_PAT[ev % len(EVICT_PAT)]
                    ev += 1
                    if eng == "dve":
                        nc.vector.tensor_scalar(
                            dst, ps[:], inv_s, None, op0=mybir.AluOpType.mult
                        )
                    elif eng == "act":
                        nc.scalar.activation(dst, ps[:], A.Copy, scale=inv_s)
                    else:
                        nc.gpsimd.tensor_scalar(
                            dst, ps[:], inv_s, None, op0=mybir.AluOpType.mult
                        )
                r0 = d["out_row0"] + g0 * 128
                nc.scalar.dma_start(
                    out_d[r0 : r0 + gn * 128, :].rearrange("(a p) e -> p a e", p=128),
                    stage[:].rearrange("p (a e) -> p a e", e=E),
                )

    nc.compile()
    _CACHE[key] = nc
    return nc


def kernel(**inputs):
    from concourse.bass_utils import run_bass_kernel_spmd

    value = np.asarray(inputs["value"], np.int64)
    depth = np.asarray(inputs["depth"], np.int64)
    position = np.asarray(inputs["position"], np.int64)
    params = {k: np.asarray(v, np.float32) for k, v in inputs.items() if "emb" in k or "conv" in k}

    tabs = _build_tables(params)

    # B per-core rows 0..71 (l0+l1 per-token sums) + l2 table + dep uniques
    dep2_uniq = np.unique(depth[:, 72:584])
    dep2_rows = np.asarray(params["dep_emb_2"], np.float32)[dep2_uniq]
    assert 264 + len(dep2_uniq) <= 512
    b_rows_percore = []
    for b in range(BATCH):
        r01 = np.zeros((72, E), np.float32)
        for l, (lo, hi) in ((0, (0, 8)), (1, (8, 72))):
            v = value[b, lo:hi]
            p = position[b, lo:hi]
            dd = depth[b, lo:hi]
            pe = np.asarray(params[f"pos_emb_{l}"], np.float32)
            r01[lo:hi] = (
                np.asarray(params[f"val_emb_{l}"], np.float32)[v]
                + pe[0][p[:, 0]]
                + pe[1][p[:, 1]]
                + pe[2][p[:, 2]]
                + np.asarray(params[f"dep_emb_{l}"], np.float32)[dd]
            )
        b_rows_percore.append(
            np.concatenate([r01, tabs["B_l2"], dep2_rows], 0)
        )

    # per-layer scales (shared across cores -> compiled immediates)
    absmax = [
        max(float(np.abs(r).max()) for r in b_rows_percore),
        float(np.abs(tabs["L3"]).max()),
        float(np.abs(tabs["L4"]).max()),
    ]
    S = [2.0 ** np.floor(np.log2(0.9 * F8_MAX / a)) for a in absmax]
    inv_s = tuple(float(1.0 / s) for s in S)

    nc = _get_nc(inv_s)

    # shared table planes (L3, L4)
    tbh_shared = np.zeros((128, NCH * E), F8)
    tbl_shared = np.zeros((128, NCH * E), F8)
    for li, name in ((1, "L3"), (2, "L4")):
        d = _L[li]
        hi, lo = _quant_hilo(_pack_chunks(tabs[name], d["nch"]), S[li])
        tbh_shared[:, d["cb"] * E : (d["cb"] + d["nch"]) * E] = hi
        tbl_shared[:, d["cb"] * E : (d["cb"] + d["nch"]) * E] = lo

    in_maps = []
    for b in range(BATCH):
        tbh = tbh_shared.copy()
        tbl = tbl_shared.copy()
        hi, lo = _quant_hilo(_pack_chunks(b_rows_percore[b], _L[0]["nch"]), S[0])
        tbh[:, : _L[0]["nch"] * E] = hi
        tbl[:, : _L[0]["nch"] * E] = lo
        mh = _build_mh(value, depth, position, b, dep2_uniq).view(F8)
        in_maps.append({"mh": mh, "tbh": tbh, "tbl": tbl})

    res = run_bass_kernel_spmd(nc, in_maps, list(range(BATCH)))
    outs = []
    for b in range(BATCH):
        o = np.asarray(res.results[b]["out"]).astype(np.float32)
        outs.append(np.concatenate([o[0:584], o[640:1664], o[1664:5760]], 0))
    return np.stack(outs)


# revision 8
# speedup vs baseline: 2.6272x; 1.0047x over previous
"""Trainium2 Bass kernel for nn_CompositeEmbeddingA (octree composite embedding).

Per sample (1 sample per NeuronCore, batch=8 over 8 cores):
  layers 0-2 (depths 1-3): x = val_emb[v] + pos0[p0] + pos1[p1] + pos2[p2] + dep_emb[d]
  layers 3-4: same sum w/o dep, then Conv1d(E,E,kernel=stride=k), k=4 (l3) / 8 (l4)

Design (v2): every layer is out = MultiHot @ Table on the PE, with
  - conv folded into the tables host-side (per tap j, T_j = concat(tables) @ w[:,:,j].T),
  - the multi-hot matrices built ON THE HOST and shipped as fp8 (exact 0/1),
    so the PE does only the main matmuls and DVE/ACT only PSUM evictions,
  - fp8e4m3 DoubleRow matmuls: 2 chunks of 128 table rows contracted per
    instruction at 0.5 cycles/row; tables stored as scaled hi+lo fp8 pairs
    (quantization residual correction) and the eviction rescales by 1/S,
  - layers 0+1 (72 tokens) use one precomputed row per token instead of
    table rows (host computes those 72 sums directly),
  - output written as bf16 and upcast on the host.
"""

import sys

for _p in ("/opt/trn_rl_repo",):
    if _p not in sys.path:
        sys.path.insert(0, _p)

import numpy as np
import ml_dtypes

E = 256
BATCH = 8
LAYER_SIZES = (8, 64, 512, 4096, 32768)
CONV_SIZE = {3: 4, 4: 8}
S_TOTAL = sum(LAYER_SIZES)

F8 = ml_dtypes.float8_e4m3
BF16 = ml_dtypes.bfloat16
F8_ONE = np.asarray(1.0, F8).view(np.uint8).item()
F8_MAX = float(ml_dtypes.finfo(F8).max)

# virtual layers: B = real layers 0-2 merged; L3/L4 conv layers.
#   B: 584 out tokens padded to 640 (5 ttiles); rows = 72 per-token rows
#      (l0+l1) + l2 table (3 val + 189 pos + dep uniques) -> 4 chunks, 2 pairs
#   L3: 1024 tokens (8 tt); 4 taps x 192 rows = 768 -> 6 chunks, 3 pairs
#   L4: 4096 tokens (32 tt); 8 taps x 192 rows = 1536 -> 12 chunks, 6 pairs
_L = [
    dict(name="B", T=584, Tp=640, ntt=5, nch=4),
    dict(name="L3", T=1024, Tp=1024, ntt=8, nch=6),
    dict(name="L4", T=4096, Tp=4096, ntt=32, nch=12),
]
_mhb = 0
_cb = 0
_orow = 0
for _d in _L:
    _d["mh_base"] = _mhb
    _d["cb"] = _cb
    _d["out_row0"] = _orow
    _d["npairs"] = _d["nch"] // 2
    _mhb += _d["ntt"] * _d["nch"] * 128
    _cb += _d["nch"]
    _orow += _d["Tp"]
NCH = _cb
MH_TOTAL = _mhb
OUT_ROWS = _orow  # 5760

# schedule knobs
MH_PIECES = {"B": (5,), "L3": (4, 4), "L4": (4, 4, 4, 4, 4, 4, 4, 2, 1, 1)}
STORE_GROUPS = {"B": (5,), "L3": (8,), "L4": (8, 8, 8, 4, 2, 2)}
EVICT_PAT = ("dve", "act")  # round robin eviction engines (gpsimd can't read PSUM)


def _build_tables(params):
    """Folded f32 tables per virtual layer (core-independent parts).

    Returns {layer_name: rows [nrows, E] f32}, without B's per-token rows
    (those are per-core, rows 0..71 of B).
    """
    out = {}
    # B: l2 table rows at offset 72: 3 val + 189 pos + dep uniques (built later)
    v2 = np.asarray(params["val_emb_2"], np.float32)[1:4]
    pe2 = np.asarray(params["pos_emb_2"], np.float32)
    out["B_l2"] = np.concatenate([v2, pe2[0][1:64], pe2[1][1:64], pe2[2][1:64]], 0)
    for name, l in (("L3", 3), ("L4", 4)):
        k = CONV_SIZE[l]
        w = np.asarray(params[f"conv_w_{l}"], np.float32)
        b = np.asarray(params[f"conv_b_{l}"], np.float32)
        pe = np.asarray(params[f"pos_emb_{l}"], np.float32)
        base = np.concatenate(
            [
                np.asarray(params[f"val_emb_{l}"], np.float32)[1:4],
                pe[0][1:64],
                pe[1][1:64],
                pe[2][1:64],
            ],
            0,
        )  # [192, E]
        taps = []
        for j in range(k):
            f = base @ w[:, :, j].T
            if j == 0:
                f[:3] += b  # bias fires exactly once per token via the val row
            taps.append(f)
        out[name] = np.concatenate(taps, 0)  # [192k, E]
    return out


def _pack_chunks(rows, nch):
    """[nrows<=nch*128, E] f32 -> [128, nch*E] (partition = row-within-chunk)."""
    buf = np.zeros((nch * 128, E), np.float32)
    buf[: rows.shape[0]] = rows
    return np.ascontiguousarray(
        buf.reshape(nch, 128, E).transpose(1, 0, 2)
    ).reshape(128, nch * E)


def _quant_hilo(packed, S):
    hi = (packed * S).astype(F8)
    lo = (packed * S - hi.astype(np.float32)).astype(F8)
    return hi, lo


def _build_mh(value, depth, position, b, dep2_uniq):
    """Host-built multi-hot for core b: [128, MH_TOTAL] uint8 (fp8 bits)."""
    pieces = []

    def emit(r_ids, t_ids, d):
        M = np.zeros(d["nch"] * 128 * d["Tp"], np.uint8)
        M[r_ids * d["Tp"] + t_ids] = F8_ONE
        M = (
            M.reshape(d["nch"], 128, d["ntt"], 128)
            .transpose(1, 2, 0, 3)
            .reshape(128, -1)
        )
        pieces.append(M)

    # --- B ---
    d = _L[0]
    t01 = np.arange(72)
    v2 = value[b, 72:584]
    p2 = position[b, 72:584]
    d2 = depth[b, 72:584]
    t2 = np.arange(72, 584)
    dep_rows = 264 + np.searchsorted(dep2_uniq, d2)
    r_ids = np.concatenate(
        [
            t01,
            72 + (v2 - 1),
            75 + (p2[:, 0] - 1),
            138 + (p2[:, 1] - 1),
            201 + (p2[:, 2] - 1),
            dep_rows,
        ]
    )
    t_ids = np.concatenate([t01, t2, t2, t2, t2, t2])
    emit(r_ids, t_ids, d)

    # --- conv layers ---
    lo = 584
    for d, l in ((_L[1], 3), (_L[2], 4)):
        k = CONV_SIZE[l]
        T = d["T"]
        v = value[b, lo : lo + T * k].reshape(T, k)
        p = position[b, lo : lo + T * k].reshape(T, k, 3)
        t = np.broadcast_to(np.arange(T)[:, None], (T, k))
        jb = np.broadcast_to(np.arange(k)[None, :] * 192, (T, k))
        r_ids = np.concatenate(
            [
                (jb + v - 1).ravel(),
                (jb + 3 + p[:, :, 0] - 1).ravel(),
                (jb + 66 + p[:, :, 1] - 1).ravel(),
                (jb + 129 + p[:, :, 2] - 1).ravel(),
            ]
        )
        t_ids = np.concatenate([t.ravel()] * 4)
        emit(r_ids, t_ids, d)
        lo += T * k

    return np.concatenate(pieces, axis=1)


_CACHE = {}


def _get_nc(inv_scales):
    key = ("v2.1", tuple(inv_scales))
    if key in _CACHE:
        return _CACHE[key]

    import concourse.bass as bass
    import concourse.tile as tile
    from concourse import bacc, mybir
    from contextlib import ExitStack

    f32 = mybir.dt.float32
    bf16 = mybir.dt.bfloat16
    f8 = mybir.dt.float8e4
    A = mybir.ActivationFunctionType
    DR = mybir.MatmulPerfMode.DoubleRow

    nc = bacc.Bacc(trn_type="TRN2", target_bir_lowering=False, debug=False)
    mh_d = nc.dram_tensor("mh", [128, MH_TOTAL], f8, kind="ExternalInput").ap()
    # per layer: nch hi chunks then nch lo chunks, contiguous -> 1 DMA/layer
    tb_d = nc.dram_tensor("tb", [128, 2 * NCH * E], f8, kind="ExternalInput").ap()
    out_d = nc.dram_tensor("out", [OUT_ROWS, E], bf16, kind="ExternalOutput").ap()

    with tile.TileContext(nc) as tc, ExitStack() as ctx:
        cpool = ctx.enter_context(tc.tile_pool(name="const", bufs=1))
        pspool = ctx.enter_context(
            tc.tile_pool(name="ps", bufs=8, space=bass.MemorySpace.PSUM)
        )
        spool = ctx.enter_context(tc.tile_pool(name="stage", bufs=1))

        tb_t = cpool.tile([128, 2 * NCH * E], f8, tag="tb")
        mh_t = cpool.tile([128, MH_TOTAL], f8, tag="mh")

        # loads in consumption order (SP queue)
        for d in _L:
            ca, cb_ = 2 * d["cb"] * E, 2 * (d["cb"] + d["nch"]) * E
            nc.sync.dma_start(tb_t[:, ca:cb_], tb_d[:, ca:cb_])
            a = d["mh_base"]
            for ptt in MH_PIECES[d["name"]]:
                bnd = a + ptt * d["nch"] * 128
                nc.sync.dma_start(mh_t[:, a:bnd], mh_d[:, a:bnd])
                a = bnd

        # compute
        ev = 0
        st = 0
        for li, d in enumerate(_L):
            inv_s = inv_scales[li]
            g0 = 0
            for gn in STORE_GROUPS[d["name"]]:
                stage = spool.tile([128, gn * E], bf16, tag=f"st{li}g{g0}")
                for ti in range(gn):
                    tt = g0 + ti
                    ps = pspool.tile([128, E], f32, tag="ps")
                    nmm = 2 * d["npairs"]
                    i = 0
                    for q in range(d["npairs"]):
                        ma = d["mh_base"] + (tt * d["nch"] + 2 * q) * 128
                        mh_ap = mh_t[:, ma : ma + 256].rearrange(
                            "p (two m) -> p two m", two=2
                        )
                        for hl in range(2):
                            ta = (2 * d["cb"] + hl * d["nch"] + 2 * q) * E
                            nc.tensor.matmul(
                                ps[:],
                                mh_ap,
                                tb_t[:, ta : ta + 2 * E].rearrange(
                                    "p (two e) -> p two e", two=2
                                ),
                                start=(i == 0),
                                stop=(i == nmm - 1),
                                perf_mode=DR,
                            )
                            i += 1
                    dst = stage[:, ti * E : (ti + 1) * E]
                    eng = EVICT_PAT[ev % len(EVICT_PAT)]
                    ev += 1
                    if eng == "dve":
                        nc.vector.tensor_scalar(
                            dst, ps[:], inv_s, None, op0=mybir.AluOpType.mult
                        )
                    else:
                        nc.scalar.activation(dst, ps[:], A.Copy, scale=inv_s)
                r0 = d["out_row0"] + g0 * 128
                seng = nc.scalar if st % 2 == 0 else nc.gpsimd
                st += 1
                seng.dma_start(
                    out_d[r0 : r0 + gn * 128, :].rearrange("(a p) e -> p a e", p=128),
                    stage[:].rearrange("p (a e) -> p a e", e=E),
                )
                g0 += gn

    nc.compile()
    _CACHE[key] = nc
    return nc


def kernel(**inputs):
    from concourse.bass_utils import run_bass_kernel_spmd

    value = np.asarray(inputs["value"], np.int64)
    depth = np.asarray(inputs["depth"], np.int64)
    position = np.asarray(inputs["position"], np.int64)
    params = {k: np.asarray(v, np.float32) for k, v in inputs.items() if "emb" in k or "conv" in k}

    tabs = _build_tables(params)

    # B per-core rows 0..71 (l0+l1 per-token sums) + l2 table + dep uniques
    dep2_uniq = np.unique(depth[:, 72:584])
    dep2_rows = np.asarray(params["dep_emb_2"], np.float32)[dep2_uniq]
    assert 264 + len(dep2_uniq) <= 512
    b_rows_percore = []
    for b in range(BATCH):
        r01 = np.zeros((72, E), np.float32)
        for l, (lo, hi) in ((0, (0, 8)), (1, (8, 72))):
            v = value[b, lo:hi]
            p = position[b, lo:hi]
            dd = depth[b, lo:hi]
            pe = np.asarray(params[f"pos_emb_{l}"], np.float32)
            r01[lo:hi] = (
                np.asarray(params[f"val_emb_{l}"], np.float32)[v]
                + pe[0][p[:, 0]]
                + pe[1][p[:, 1]]
                + pe[2][p[:, 2]]
                + np.asarray(params[f"dep_emb_{l}"], np.float32)[dd]
            )
        b_rows_percore.append(
            np.concatenate([r01, tabs["B_l2"], dep2_rows], 0)
        )

    # per-layer scales (shared across cores -> compiled immediates)
    absmax = [
        max(float(np.abs(r).max()) for r in b_rows_percore),
        float(np.abs(tabs["L3"]).max()),
        float(np.abs(tabs["L4"]).max()),
    ]
    S = [2.0 ** np.floor(np.log2(0.9 * F8_MAX / a)) for a in absmax]
    inv_s = tuple(float(1.0 / s) for s in S)

    nc = _get_nc(inv_s)

    # table tensor: per layer [hi chunks | lo chunks] contiguous (1 DMA/layer)
    tb_shared = np.zeros((128, 2 * NCH * E), F8)
    for li, name in ((1, "L3"), (2, "L4")):
        d = _L[li]
        hi, lo = _quant_hilo(_pack_chunks(tabs[name], d["nch"]), S[li])
        ca = 2 * d["cb"] * E
        tb_shared[:, ca : ca + d["nch"] * E] = hi
        tb_shared[:, ca + d["nch"] * E : ca + 2 * d["nch"] * E] = lo

    in_maps = []
    for b in range(BATCH):
        tb = tb_shared.copy()
        hi, lo = _quant_hilo(_pack_chunks(b_rows_percore[b], _L[0]["nch"]), S[0])
        tb[:, : _L[0]["nch"] * E] = hi
        tb[:, _L[0]["nch"] * E : 2 * _L[0]["nch"] * E] = lo
        mh = _build_mh(value, depth, position, b, dep2_uniq).view(F8)
        in_maps.append({"mh": mh, "tb": tb})

    res = run_bass_kernel_spmd(nc, in_maps, list(range(BATCH)))
    outs = []
    for b in range(BATCH):
        o = np.asarray(res.results[b]["out"]).astype(np.float32)
        outs.append(np.concatenate([o[0:584], o[640:1664], o[1664:5760]], 0))
    return np.stack(outs)


# revision 11
# speedup vs baseline: 2.6832x; 1.0213x over previous
"""Trainium2 Bass kernel for nn_CompositeEmbeddingA (octree composite embedding).

Per sample (1 sample per NeuronCore, batch=8 over 8 cores):
  layers 0-2 (depths 1-3): x = val_emb[v] + pos0[p0] + pos1[p1] + pos2[p2] + dep_emb[d]
  layers 3-4: same sum w/o dep, then Conv1d(E,E,kernel=stride=k), k=4 (l3) / 8 (l4)

Design (v2): every layer is out = MultiHot @ Table on the PE, with
  - conv folded into the tables host-side (per tap j, T_j = concat(tables) @ w[:,:,j].T),
  - the multi-hot matrices built ON THE HOST and shipped as fp8 (exact 0/1),
    so the PE does only the main matmuls and DVE/ACT only PSUM evictions,
  - fp8e4m3 DoubleRow matmuls: 2 chunks of 128 table rows contracted per
    instruction at 0.5 cycles/row; tables stored as scaled hi+lo fp8 pairs
    (quantization residual correction) and the eviction rescales by 1/S,
  - layers 0+1 (72 tokens) use one precomputed row per token instead of
    table rows (host computes those 72 sums directly),
  - output written as bf16 and upcast on the host.
"""

import sys

for _p in ("/opt/trn_rl_repo",):
    if _p not in sys.path:
        sys.path.insert(0, _p)

import numpy as np
import ml_dtypes

E = 256
BATCH = 8
LAYER_SIZES = (8, 64, 512, 4096, 32768)
CONV_SIZE = {3: 4, 4: 8}
S_TOTAL = sum(LAYER_SIZES)

F8 = ml_dtypes.float8_e4m3
BF16 = ml_dtypes.bfloat16
F8_ONE = np.asarray(1.0, F8).view(np.uint8).item()
F8_MAX = float(ml_dtypes.finfo(F8).max)

# virtual layers: B = real layers 0-2 merged; L3/L4 conv layers.
#   B: 584 out tokens padded to 640 (5 ttiles); rows = 72 per-token rows
#      (l0+l1) + l2 table (3 val + 189 pos + dep uniques) -> 4 chunks, 2 pairs
#   L3: 1024 tokens (8 tt); 4 taps x 192 rows = 768 -> 6 chunks, 3 pairs
#   L4: 4096 tokens (32 tt); 8 taps x 192 rows = 1536 -> 12 chunks, 6 pairs
_L = [
    dict(name="B", T=584, Tp=640, ntt=5, nch=4),
    dict(name="L3", T=1024, Tp=1024, ntt=8, nch=6),
    dict(name="L4", T=4096, Tp=4096, ntt=32, nch=12),
]
_mhb = 0
_cb = 0
_orow = 0
for _d in _L:
    _d["mh_base"] = _mhb
    _d["cb"] = _cb
    _d["out_row0"] = _orow
    _d["npairs"] = _d["nch"] // 2
    _mhb += _d["ntt"] * _d["nch"] * 128
    _cb += _d["nch"]
    _orow += _d["Tp"]
NCH = _cb
MH_TOTAL = _mhb
OUT_ROWS = _orow  # 5760

# schedule: (layer_index, ttile_start, n_ttiles, mh piece split) per store group,
# in processing order. L4's last-loaded group is g2 so the final store's data
# is ready well before the DMA queue drains.
SCHEDULE = [
    (0, 0, 5, (1, 4)),
    (1, 0, 8, (4, 4)),
    (2, 24, 8, (4, 4)),
    (2, 0, 8, (4, 4)),
    (2, 8, 8, (4, 4)),
    (2, 16, 8, (4, 4)),
]
EVICT_PAT = ("dve", "act")  # round robin eviction engines (gpsimd can't read PSUM)


def _build_tables(params):
    """Folded f32 tables per virtual layer (core-independent parts).

    Returns {layer_name: rows [nrows, E] f32}, without B's per-token rows
    (those are per-core, rows 0..71 of B).
    """
    out = {}
    # B: l2 table rows at offset 72: 3 val + 189 pos + dep uniques (built later)
    v2 = np.asarray(params["val_emb_2"], np.float32)[1:4]
    pe2 = np.asarray(params["pos_emb_2"], np.float32)
    out["B_l2"] = np.concatenate([v2, pe2[0][1:64], pe2[1][1:64], pe2[2][1:64]], 0)
    for name, l in (("L3", 3), ("L4", 4)):
        k = CONV_SIZE[l]
        w = np.asarray(params[f"conv_w_{l}"], np.float32)
        b = np.asarray(params[f"conv_b_{l}"], np.float32)
        pe = np.asarray(params[f"pos_emb_{l}"], np.float32)
        base = np.concatenate(
            [
                np.asarray(params[f"val_emb_{l}"], np.float32)[1:4],
                pe[0][1:64],
                pe[1][1:64],
                pe[2][1:64],
            ],
            0,
        )  # [192, E]
        taps = []
        for j in range(k):
            f = base @ w[:, :, j].T
            if j == 0:
                f[:3] += b  # bias fires exactly once per token via the val row
            taps.append(f)
        out[name] = np.concatenate(taps, 0)  # [192k, E]
    return out


def _pack_chunks(rows, nch):
    """[nrows<=nch*128, E] f32 -> [128, nch*E] (partition = row-within-chunk)."""
    buf = np.zeros((nch * 128, E), np.float32)
    buf[: rows.shape[0]] = rows
    return np.ascontiguousarray(
        buf.reshape(nch, 128, E).transpose(1, 0, 2)
    ).reshape(128, nch * E)


def _quant_hilo(packed, S):
    hi = (packed * S).astype(F8)
    lo = (packed * S - hi.astype(np.float32)).astype(F8)
    return hi, lo


def _build_mh(value, depth, position, b, dep2_uniq):
    """Host-built multi-hot for core b: [128, MH_TOTAL] uint8 (fp8 bits)."""
    pieces = []

    def emit(r_ids, t_ids, d):
        M = np.zeros(d["nch"] * 128 * d["Tp"], np.uint8)
        M[r_ids * d["Tp"] + t_ids] = F8_ONE
        M = (
            M.reshape(d["nch"], 128, d["ntt"], 128)
            .transpose(1, 2, 0, 3)
            .reshape(128, -1)
        )
        pieces.append(M)

    # --- B ---
    d = _L[0]
    t01 = np.arange(72)
    v2 = value[b, 72:584]
    p2 = position[b, 72:584]
    d2 = depth[b, 72:584]
    t2 = np.arange(72, 584)
    dep_rows = 264 + np.searchsorted(dep2_uniq, d2)
    r_ids = np.concatenate(
        [
            t01,
            72 + (v2 - 1),
            75 + (p2[:, 0] - 1),
            138 + (p2[:, 1] - 1),
            201 + (p2[:, 2] - 1),
            dep_rows,
        ]
    )
    t_ids = np.concatenate([t01, t2, t2, t2, t2, t2])
    emit(r_ids, t_ids, d)

    # --- conv layers ---
    lo = 584
    for d, l in ((_L[1], 3), (_L[2], 4)):
        k = CONV_SIZE[l]
        T = d["T"]
        v = value[b, lo : lo + T * k].reshape(T, k)
        p = position[b, lo : lo + T * k].reshape(T, k, 3)
        t = np.broadcast_to(np.arange(T)[:, None], (T, k))
        jb = np.broadcast_to(np.arange(k)[None, :] * 192, (T, k))
        r_ids = np.concatenate(
            [
                (jb + v - 1).ravel(),
                (jb + 3 + p[:, :, 0] - 1).ravel(),
                (jb + 66 + p[:, :, 1] - 1).ravel(),
                (jb + 129 + p[:, :, 2] - 1).ravel(),
            ]
        )
        t_ids = np.concatenate([t.ravel()] * 4)
        emit(r_ids, t_ids, d)
        lo += T * k

    return np.concatenate(pieces, axis=1)


_CACHE = {}


def _get_nc(inv_scales):
    key = ("v2.2", tuple(inv_scales), tuple(SCHEDULE))
    if key in _CACHE:
        return _CACHE[key]

    import concourse.bass as bass
    import concourse.tile as tile
    from concourse import bacc, mybir
    from contextlib import ExitStack

    f32 = mybir.dt.float32
    bf16 = mybir.dt.bfloat16
    f8 = mybir.dt.float8e4
    A = mybir.ActivationFunctionType
    DR = mybir.MatmulPerfMode.DoubleRow

    nc = bacc.Bacc(trn_type="TRN2", target_bir_lowering=False, debug=False)
    mh_d = nc.dram_tensor("mh", [128, MH_TOTAL], f8, kind="ExternalInput").ap()
    # per layer: nch hi chunks then nch lo chunks, contiguous -> 1 DMA/layer
    tb_d = nc.dram_tensor("tb", [128, 2 * NCH * E], f8, kind="ExternalInput").ap()
    out_d = nc.dram_tensor("out", [OUT_ROWS, E], bf16, kind="ExternalOutput").ap()

    with tile.TileContext(nc) as tc, ExitStack() as ctx:
        cpool = ctx.enter_context(tc.tile_pool(name="const", bufs=1))
        pspool = ctx.enter_context(
            tc.tile_pool(name="ps", bufs=8, space=bass.MemorySpace.PSUM)
        )
        spool = ctx.enter_context(tc.tile_pool(name="stage", bufs=1))

        tb_t = cpool.tile([128, 2 * NCH * E], f8, tag="tb")
        mh_t = cpool.tile([128, MH_TOTAL], f8, tag="mh")

        # loads in processing order (SP queue); table load before a layer's
        # first mh piece
        tb_loaded = set()
        for li, g0, gn, pieces in SCHEDULE:
            d = _L[li]
            if li not in tb_loaded:
                tb_loaded.add(li)
                ca, cb_ = 2 * d["cb"] * E, 2 * (d["cb"] + d["nch"]) * E
                nc.sync.dma_start(tb_t[:, ca:cb_], tb_d[:, ca:cb_])
            a = d["mh_base"] + g0 * d["nch"] * 128
            for ptt in pieces:
                bnd = a + ptt * d["nch"] * 128
                nc.sync.dma_start(mh_t[:, a:bnd], mh_d[:, a:bnd])
                a = bnd

        # compute
        ev = 0
        nsg = len(SCHEDULE)
        for sg, (li, g0, gn, pieces) in enumerate(SCHEDULE):
            d = _L[li]
            inv_s = inv_scales[li]
            stage = spool.tile([128, gn * E], bf16, tag=f"st{li}g{g0}")
            for ti in range(gn):
                tt = g0 + ti
                ps = pspool.tile([128, E], f32, tag="ps")
                nmm = 2 * d["npairs"]
                i = 0
                for q in range(d["npairs"]):
                    ma = d["mh_base"] + (tt * d["nch"] + 2 * q) * 128
                    mh_ap = mh_t[:, ma : ma + 256].rearrange(
                        "p (two m) -> p two m", two=2
                    )
                    for hl in range(2):
                        ta = (2 * d["cb"] + hl * d["nch"] + 2 * q) * E
                        nc.tensor.matmul(
                            ps[:],
                            mh_ap,
                            tb_t[:, ta : ta + 2 * E].rearrange(
                                "p (two e) -> p two e", two=2
                            ),
                            start=(i == 0),
                            stop=(i == nmm - 1),
                            perf_mode=DR,
                        )
                        i += 1
                dst = stage[:, ti * E : (ti + 1) * E]
                eng = EVICT_PAT[ev % len(EVICT_PAT)]
                ev += 1
                if eng == "dve":
                    nc.vector.tensor_scalar(
                        dst, ps[:], inv_s, None, op0=mybir.AluOpType.mult
                    )
                else:
                    nc.scalar.activation(dst, ps[:], A.Copy, scale=inv_s)
            r0 = d["out_row0"] + g0 * 128
            seng = nc.scalar if sg == nsg - 1 else nc.gpsimd
            seng.dma_start(
                out_d[r0 : r0 + gn * 128, :].rearrange("(a p) e -> p a e", p=128),
                stage[:].rearrange("p (a e) -> p a e", e=E),
            )

    nc.compile()
    _CACHE[key] = nc
    return nc


def kernel(**inputs):
    from concourse.bass_utils import run_bass_kernel_spmd

    value = np.asarray(inputs["value"], np.int64)
    depth = np.asarray(inputs["depth"], np.int64)
    position = np.asarray(inputs["position"], np.int64)
    params = {k: np.asarray(v, np.float32) for k, v in inputs.items() if "emb" in k or "conv" in k}

    tabs = _build_tables(params)

    # B per-core rows 0..71 (l0+l1 per-token sums) + l2 table + dep uniques
    dep2_uniq = np.unique(depth[:, 72:584])
    dep2_rows = np.asarray(params["dep_emb_2"], np.float32)[dep2_uniq]
    assert 264 + len(dep2_uniq) <= 512
    b_rows_percore = []
    for b in range(BATCH):
        r01 = np.zeros((72, E), np.float32)
        for l, (lo, hi) in ((0, (0, 8)), (1, (8, 72))):
            v = value[b, lo:hi]
            p = position[b, lo:hi]
            dd = depth[b, lo:hi]
            pe = np.asarray(params[f"pos_emb_{l}"], np.float32)
            r01[lo:hi] = (
                np.asarray(params[f"val_emb_{l}"], np.float32)[v]
                + pe[0][p[:, 0]]
                + pe[1][p[:, 1]]
                + pe[2][p[:, 2]]
                + np.asarray(params[f"dep_emb_{l}"], np.float32)[dd]
            )
        b_rows_percore.append(
            np.concatenate([r01, tabs["B_l2"], dep2_rows], 0)
        )

    # per-layer scales (shared across cores -> compiled immediates)
    absmax = [
        max(float(np.abs(r).max()) for r in b_rows_percore),
        float(np.abs(tabs["L3"]).max()),
        float(np.abs(tabs["L4"]).max()),
    ]
    S = [2.0 ** np.floor(np.log2(0.9 * F8_MAX / a)) for a in absmax]
    inv_s = tuple(float(1.0 / s) for s in S)

    nc = _get_nc(inv_s)

    # table tensor: per layer [hi chunks | lo chunks] contiguous (1 DMA/layer)
    tb_shared = np.zeros((128, 2 * NCH * E), F8)
    for li, name in ((1, "L3"), (2, "L4")):
        d = _L[li]
        hi, lo = _quant_hilo(_pack_chunks(tabs[name], d["nch"]), S[li])
        ca = 2 * d["cb"] * E
        tb_shared[:, ca : ca + d["nch"] * E] = hi
        tb_shared[:, ca + d["nch"] * E : ca + 2 * d["nch"] * E] = lo

    in_maps = []
    for b in range(BATCH):
        tb = tb_shared.copy()
        hi, lo = _quant_hilo(_pack_chunks(b_rows_percore[b], _L[0]["nch"]), S[0])
        tb[:, : _L[0]["nch"] * E] = hi
        tb[:, _L[0]["nch"] * E : 2 * _L[0]["nch"] * E] = lo
        mh = _build_mh(value, depth, position, b, dep2_uniq).view(F8)
        in_maps.append({"mh": mh, "tb": tb})

    res = run_bass_kernel_spmd(nc, in_maps, list(range(BATCH)))
    outs = []
    for b in range(BATCH):
        o = np.asarray(res.results[b]["out"]).astype(np.float32)
        outs.append(np.concatenate([o[0:584], o[640:1664], o[1664:5760]], 0))
    return np.stack(outs)
